# revision 2
# baseline (speedup 1.0000x reference)
"""Conv3D (stride (1,2,2), pad (2,3,3)) as a Bass/Tile kernel for 8 trn2 cores.

Problem: x (8,3,16,112,112) f32, weight (64,3,5,7,7), bias (64,)
      -> out (8,64,16,56,56).  Data-parallel: one batch sample per core.

Device strategy (per core, per output depth od):
  The contraction over (c=3, kw=7, kd=5) = 105 terms is packed on the PE
  partition axis (plus a constant-ones row carrying the bias), and the
  remaining kernel dim kh=7 is a PSUM accumulation loop.  The stride-2
  output-row walk rides a strided access pattern; the kw (stride-2 cols)
  and kd (depth) shifts cannot vary per-partition in one AP, so the host
  pre-builds a duplicated layout

     R[od, p=(c,kw,kd), hp, j] = Xpad[c, od+kd, hp, 2*j+kw]   (+ ones row)

  and the kernel streams matmuls
     psum[o, oh, j] (+)= sum_p W[p, kh, o] * R[od, p, 2*oh+kh, j]
  as fp32r (fp22 multiplies, fp32 accumulate, full PE rate at N=448).
"""

import numpy as np

import concourse.bass as bass
import concourse.mybir as mybir
import concourse.tile as tile
from concourse import bacc
from concourse.bass_utils import run_bass_kernel_spmd

N, C, D, H, W = 8, 3, 16, 112, 112
O, KD, KH, KW = 64, 5, 7, 7
PD, PH, PW = 2, 3, 3
OD, OH, OW = 16, 56, 56
KP = C * KW * KD          # 105 contraction rows
NP = KP + 1               # + ones row for bias
HP = H + 2 * PH           # 118 padded input rows
OHB = 8                   # output rows per psum tile
OHC = OH // OHB           # 7 psum tiles per od

_CACHE = {}
LAST_RUN = None


def _build_bass():
    nc = bacc.Bacc("TRN2", target_bir_lowering=False, debug=False, num_devices=N)
    r = nc.dram_tensor("r", [OD, NP, HP, OW], mybir.dt.float32r, kind="ExternalInput")
    w = nc.dram_tensor("w", [NP, KH, O], mybir.dt.float32r, kind="ExternalInput")
    out = nc.dram_tensor("out", [O, OD, OH, OW], mybir.dt.float32, kind="ExternalOutput")
    f32r = mybir.dt.float32r

    with tile.TileContext(nc) as tc:
        with (
            tc.tile_pool(name="wp", bufs=1) as wp,
            tc.tile_pool(name="sp", bufs=3) as sp,
            tc.tile_pool(name="op", bufs=3) as op,
            tc.tile_pool(name="pp", bufs=8, space=bass.MemorySpace.PSUM) as pp,
        ):
            wt = wp.tile([NP, KH, O], f32r)
            nc.sync.dma_start(wt[:], w[:])
            for od in range(OD):
                s = sp.tile([NP, HP, OW], f32r)
                nc.sync.dma_start(s[:], r[od])
                ob = op.tile([O, OH, OW], mybir.dt.float32)
                for t in range(OHC):
                    ps = pp.tile([O, OHB, OW], mybir.dt.float32)
                    for kh in range(KH):
                        # rows hp = 2*oh + kh for oh in [OHB*t, OHB*(t+1))
                        base = 2 * OHB * t + kh
                        rhs = s[0:NP, base : base + 2 * OHB : 2, :]
                        nc.tensor.matmul(
                            ps[:],
                            wt[0:NP, kh, :],
                            rhs,
                            start=(kh == 0),
                            stop=(kh == KH - 1),
                        )
                    nc.scalar.copy(ob[0:O, OHB * t : OHB * (t + 1), :], ps[:])
                nc.sync.dma_start(out[0:O, od, :, :], ob[:])
    nc.compile()
    return nc


def _host_pack(x, weight, bias):
    """Build the pre-shifted rhs volume R per sample and the weight tiles."""
    xf = np.ascontiguousarray(x, dtype=np.float32)
    xp = np.zeros((N, C, D + 2 * PD, HP, W + 2 * PW), np.float32)
    xp[:, :, PD : PD + D, PH : PH + H, PW : PW + W] = xf

    R = np.empty((N, OD, NP, HP, OW), np.float32)
    p = 0
    for c in range(C):
        for kw in range(KW):
            for kd in range(KD):
                # R[n, od, p, hp, j] = xp[n, c, od+kd, hp, 2*j+kw]
                R[:, :, p] = xp[:, c, kd : kd + OD, :, kw : kw + 2 * OW : 2].transpose(
                    0, 1, 2, 3
                )
                p += 1
    R[:, :, KP] = 1.0

    # Wt[p=(c,kw,kd), kh, o]; ones row carries bias on kh=0 only.
    Wt = np.zeros((NP, KH, O), np.float32)
    Wt[:KP] = (
        np.asarray(weight, np.float32)
        .transpose(1, 4, 2, 3, 0)  # [C, KW, KD, KH, O]
        .reshape(KP, KH, O)
    )
    Wt[KP, 0] = np.asarray(bias, np.float32)
    return R, Wt


def kernel(x, weight, bias):
    global LAST_RUN
    if "nc" not in _CACHE:
        _CACHE["nc"] = _build_bass()
    nc = _CACHE["nc"]

    R, Wt = _host_pack(x, weight, bias)
    in_maps = [{"r": R[n], "w": Wt} for n in range(N)]
    res = run_bass_kernel_spmd(nc, in_maps, core_ids=list(range(N)))
    LAST_RUN = res
    out = np.stack([res.results[n]["out"] for n in range(N)], axis=0)
    return out.astype(np.float32, copy=False)
